# revision 7
# baseline (speedup 1.0000x reference)
"""Trainium2 Bass kernel for BaseNeuron degree-feature spiking forward.

Computes, for dv [500000, 128] f32, binned_degree [500000] i32,
v_threshold [20, 128] f32, tau, alpha scalars:

    v      = dv / tau
    thresh = v_threshold[binned_degree]          # per-node row gather
    spike  = (v - thresh > 0).astype(f32)        # heaviside forward
    sums   = segment_sum(spike, binned_degree)   # [20, 128]
    counts = bincount(binned_degree)             # [20]
    v_th_new = where(counts>0, 0.2*sums/max(counts,1) + 0.8*v_threshold,
                     v_threshold)

Returns (spike, v_th_new) like the reference.

Strategy: shard nodes across 8 NeuronCores (62500 nodes each, padded to
63488 = 496 tiles of 128). Per core the device computes spike and the
per-bin partial sums (as sumsT [128 s, 20 b] in PSUM); the tiny [20,128]
cross-core reduction + EMA update run on host.

Three program variants, picked from the actual v_threshold values:
  - uniform: all 20 threshold rows identical (the module's init state) ->
    thresh is bin-independent; compare against one broadcast row, no gather.
  - bf16 gather: rows differ but are bf16-exact -> batched one-hot matmul
    thresh[128,512] = onehotT[80,128].T @ blockdiag(vth)[80,512] in bf16.
  - f32 gather: same with f32 operands (exact for any f32 table).

Per 4-tile group (512 nodes): one fused VectorE op
    spike_fp8 = (dv * (1/tau)) > thresh
(fp8e4m3 is exact for {0,1}); per 128-node tile one TensorE matmul
    sumsT[128,20] += spike[128,128].T @ onehot[128,20]     (fp8, FWL)
accumulated in a persistent PSUM bank over all 496 tiles.

The [128,20] one-hot tiles are generated on the idle GPSIMD engine from a
small bf16 bin-index array (binT) via a broadcast is_equal, one op per
chunk, instead of being DMAed from HBM.

dv and spike live in DRAM pre-permuted as [128, TILES*128] (partition-
contiguous rows -> large DMA descriptors); the host does the permutes and
widens the fp8 spike back to f32.
"""

import os
import numpy as np

import concourse.bass as bass
import concourse.bacc as bacc
import concourse.mybir as mybir
from concourse.tile import TileContext
from concourse import bass_utils

F32 = mybir.dt.float32
BF16 = mybir.dt.bfloat16
F8 = mybir.dt.float8e4
ALU = mybir.AluOpType
NP_BF16 = mybir.dt.np(BF16)
NP_F8 = mybir.dt.np(F8)

# Problem geometry (hardcoded per contract).
N_FULL = 500000
S = 128
BINS = 20
CORES = 8
SHARD = N_FULL // CORES          # 62500 real nodes per core
TILES = 496                      # node tiles of 128 per core (padded)
PAD_ROWS = TILES * 128           # 63488 rows fed to each core
GROUP = 4                        # tiles per compare group (free dim 512)
NGROUPS = TILES // GROUP         # 124
CHUNK = 16                       # tiles per DMA chunk (1 MiB of dv)
NCHUNKS = TILES // CHUNK         # 31
GROUPS_PER_CHUNK = CHUNK // GROUP
GAMMA = np.float32(0.2)

_PROGRAM_CACHE = {}
LAST_RESULTS = None              # BassKernelResults of the most recent run


def _build_program(inv_tau: float, mode: str):
    """mode: 'uniform' | 'bf16' | 'f32'"""
    GDT = F32 if mode == "f32" else BF16
    nc = bacc.Bacc(
        "TRN2",
        target_bir_lowering=False,
        debug=False,
        num_devices=CORES,
    )

    dv = nc.dram_tensor("dv", [128, TILES * S], F32, kind="ExternalInput").ap()
    binT = nc.dram_tensor("binT", [128, TILES], BF16, kind="ExternalInput").ap()
    iota = nc.dram_tensor("iota", [128, BINS], BF16, kind="ExternalInput").ap()
    if mode == "uniform":
        vthb = nc.dram_tensor(
            "vthb", [128, GROUP * S], F32, kind="ExternalInput"
        ).ap()
    else:
        ohT = nc.dram_tensor(
            "ohT", [GROUP * BINS, NGROUPS * 128], GDT, kind="ExternalInput"
        ).ap()
        vth4 = nc.dram_tensor(
            "vth4", [GROUP * BINS, GROUP * S], GDT, kind="ExternalInput"
        ).ap()
    spike = nc.dram_tensor(
        "spike", [128, TILES * S], F8, kind="ExternalOutput"
    ).ap()
    sumsT = nc.dram_tensor("sumsT", [S, BINS], F32, kind="ExternalOutput").ap()

    with TileContext(nc) as tc:
        with (
            tc.tile_pool(name="const", bufs=1) as cpool,
            tc.tile_pool(name="io", bufs=8) as iopool,
            tc.tile_pool(name="ohp", bufs=4) as ohpool,
            tc.tile_pool(name="psum", bufs=4, space="PSUM") as ppool,
            tc.tile_pool(name="acc", bufs=1, space="PSUM") as apool,
        ):
            # Constants ride the scalar-engine HWDGE ring so the first dv
            # chunk can start immediately on the sync ring.
            binT_sb = cpool.tile([128, TILES], BF16)
            nc.scalar.dma_start(out=binT_sb[:], in_=binT)
            iota_sb = cpool.tile([128, BINS], BF16)
            nc.scalar.dma_start(out=iota_sb[:], in_=iota)
            iota_b = iota_sb[:].rearrange("p (o b) -> p o b", o=1).to_broadcast(
                [128, CHUNK, BINS]
            )
            if mode == "uniform":
                vthb_sb = cpool.tile([128, GROUP * S], F32)
                nc.scalar.dma_start(out=vthb_sb[:], in_=vthb)
            else:
                ohT_sb = cpool.tile([GROUP * BINS, NGROUPS * 128], GDT)
                nc.scalar.dma_start(out=ohT_sb[:], in_=ohT)
                vth4_sb = cpool.tile([GROUP * BINS, GROUP * S], GDT)
                nc.scalar.dma_start(out=vth4_sb[:], in_=vth4)

            sumsT_ps = apool.tile([S, BINS], F32)

            for c in range(NCHUNKS):
                dv_sb = iopool.tile([128, CHUNK * S], F32, tag="dv")
                nc.sync.dma_start(
                    out=dv_sb[:], in_=dv[:, c * CHUNK * S : (c + 1) * CHUNK * S]
                )
                spike_sb = iopool.tile([128, CHUNK * S], F8, tag="spike")

                # one-hot tiles for this chunk (walrus rejects this op on
                # GPSIMD's Pool engine, so it rides the vector engine's slack)
                oh_sb = ohpool.tile([128, CHUNK * BINS], F8, tag="oh")
                nc.vector.tensor_tensor(
                    out=oh_sb[:].rearrange("p (t b) -> p t b", b=BINS),
                    in0=iota_b,
                    in1=binT_sb[:, c * CHUNK : (c + 1) * CHUNK].to_broadcast(
                        [128, CHUNK, BINS]
                    ),
                    op=ALU.is_equal,
                )

                for g in range(GROUPS_PER_CHUNK):
                    gg = c * GROUPS_PER_CHUNK + g        # global group index
                    lo, hi = g * GROUP * S, (g + 1) * GROUP * S
                    if mode == "uniform":
                        thr = vthb_sb[:]
                    else:
                        th_ps = ppool.tile([128, GROUP * S], F32, tag="th")
                        nc.tensor.matmul(
                            th_ps[:],
                            ohT_sb[:, gg * 128 : (gg + 1) * 128],
                            vth4_sb[:],
                            start=True,
                            stop=True,
                        )
                        thr = th_ps[:]
                    nc.vector.scalar_tensor_tensor(
                        out=spike_sb[:, lo:hi],
                        in0=dv_sb[:, lo:hi],
                        scalar=float(inv_tau),
                        in1=thr,
                        op0=ALU.mult,
                        op1=ALU.is_gt,
                    )
                    for t in range(GROUP):
                        tt = g * GROUP + t               # tile within chunk
                        gt = gg * GROUP + t              # global tile index
                        nc.tensor.matmul(
                            sumsT_ps[:],
                            spike_sb[:, tt * S : (tt + 1) * S],
                            oh_sb[:, tt * BINS : (tt + 1) * BINS],
                            start=(gt == 0),
                            stop=(gt == TILES - 1),
                            skip_group_check=True,
                        )

                nc.scalar.dma_start(
                    out=spike[:, c * CHUNK * S : (c + 1) * CHUNK * S],
                    in_=spike_sb[:],
                )

            sumsT_sb = cpool.tile([S, BINS], F32)
            nc.scalar.copy(out=sumsT_sb[:], in_=sumsT_ps[:])
            nc.sync.dma_start(out=sumsT, in_=sumsT_sb[:])

    nc.compile()
    return nc


def _pick_mode(vth: np.ndarray) -> str:
    if bool((vth == vth[0:1]).all()):
        return "uniform"
    if bool((vth.astype(NP_BF16).astype(np.float32) == vth).all()):
        return "bf16"
    return "f32"


def _host_inputs(dv, binned_degree, v_threshold, mode: str):
    """Build the 8 per-core input maps (permuted shards + bin indices)."""
    dv = np.ascontiguousarray(dv, dtype=np.float32)
    bins = np.asarray(binned_degree).astype(np.int64)
    vth = np.asarray(v_threshold, dtype=np.float32)
    gdt = np.float32 if mode == "f32" else NP_BF16

    iota = np.ascontiguousarray(
        np.tile(np.arange(BINS), (128, 1))
    ).astype(NP_BF16)
    if mode == "uniform":
        vthb = np.ascontiguousarray(
            np.tile(vth[0], (128, GROUP))
        ).astype(np.float32)                             # [128, GROUP*S]
    else:
        vth4 = np.zeros((GROUP * BINS, GROUP * S), dtype=np.float32)
        for t in range(GROUP):
            vth4[t * BINS : (t + 1) * BINS, t * S : (t + 1) * S] = vth
        vth4 = vth4.astype(gdt)

    in_maps = []
    for c in range(CORES):
        start = c * SHARD
        stop = start + PAD_ROWS
        if stop <= N_FULL:
            shard = dv[start:stop]
        else:
            shard = np.zeros((PAD_ROWS, S), dtype=np.float32)
            shard[: N_FULL - start] = dv[start:]
        # permute to [partition, tile*S]: dv2[p, t*S+s] = shard[t*128+p, s]
        dv2 = np.ascontiguousarray(
            shard.reshape(TILES, 128, S).transpose(1, 0, 2)
        ).reshape(128, TILES * S)

        b = np.full(PAD_ROWS, -1, dtype=np.int64)
        b[:SHARD] = bins[start : start + SHARD]          # only own rows counted
        binT = np.ascontiguousarray(
            b.reshape(TILES, 128).T
        ).astype(NP_BF16)                                # [128, TILES]
        m = {"dv": dv2, "binT": binT, "iota": iota}
        if mode == "uniform":
            m["vthb"] = vthb
        else:
            onehot = b.reshape(TILES, 128).T[:, :, None] == np.arange(BINS)
            m["ohT"] = np.ascontiguousarray(
                onehot.reshape(128, NGROUPS, GROUP, BINS)
                .transpose(2, 3, 1, 0)                   # [GROUP, BINS, ngrp, 128]
                .reshape(GROUP * BINS, NGROUPS * 128)
                .astype(gdt)
            )
            m["vth4"] = vth4
        in_maps.append(m)
    return in_maps


def kernel(dv, binned_degree, v_threshold, tau, alpha):
    global LAST_RESULTS
    inv_tau = 1.0 / float(tau)

    vth = np.asarray(v_threshold, dtype=np.float32)
    mode = _pick_mode(vth)

    key = (inv_tau, mode)
    if key not in _PROGRAM_CACHE:
        _PROGRAM_CACHE[key] = _build_program(inv_tau, mode)
    nc = _PROGRAM_CACHE[key]

    in_maps = _host_inputs(dv, binned_degree, v_threshold, mode)

    trace = bool(int(os.environ.get("KERNEL_TRACE", "0")))
    res = bass_utils.run_bass_kernel_spmd(
        nc, in_maps, core_ids=list(range(CORES)), trace=trace
    )
    LAST_RESULTS = res

    spike = np.empty((N_FULL, S), dtype=np.float32)
    for c in range(CORES):
        sp2 = res.results[c]["spike"]                    # [128, TILES*S] fp8
        sp = sp2.reshape(128, TILES, S).transpose(1, 0, 2).reshape(PAD_ROWS, S)
        spike[c * SHARD : (c + 1) * SHARD] = sp[:SHARD]  # widen fp8 -> f32

    sums = np.zeros((BINS, S), dtype=np.float32)
    for c in range(CORES):
        sums += res.results[c]["sumsT"].T

    bins = np.asarray(binned_degree).astype(np.int64)
    counts = np.bincount(bins, minlength=BINS).astype(np.float32)
    mean = sums / np.maximum(counts, np.float32(1.0))[:, None]
    v_th_new = np.where(
        (counts > 0)[:, None],
        GAMMA * mean + (np.float32(1.0) - GAMMA) * vth,
        vth,
    ).astype(np.float32)

    return spike, v_th_new


# revision 10
# speedup vs baseline: 1.0646x; 1.0646x over previous
"""Trainium2 Bass kernel for BaseNeuron degree-feature spiking forward.

Computes, for dv [500000, 128] f32, binned_degree [500000] i32,
v_threshold [20, 128] f32, tau, alpha scalars:

    v      = dv / tau
    thresh = v_threshold[binned_degree]          # per-node row gather
    spike  = (v - thresh > 0).astype(f32)        # heaviside forward
    sums   = segment_sum(spike, binned_degree)   # [20, 128]
    counts = bincount(binned_degree)             # [20]
    v_th_new = where(counts>0, 0.2*sums/max(counts,1) + 0.8*v_threshold,
                     v_threshold)

Returns (spike, v_th_new) like the reference.

Strategy: shard nodes across 8 NeuronCores (62500 nodes each, padded to
63488 = 496 tiles of 128). Per core the device computes spike and the
per-bin partial sums (as sumsT [128 s, 20 b] in PSUM); the tiny [20,128]
cross-core reduction + EMA update run on host.

Three program variants, picked from the actual v_threshold values:
  - uniform: all 20 threshold rows identical (the module's init state) ->
    thresh is bin-independent; compare against one broadcast row, no gather.
  - bf16 gather: rows differ but are bf16-exact -> batched one-hot matmul
    thresh[128,512] = onehotT[80,128].T @ blockdiag(vth)[80,512] in bf16.
  - f32 gather: same with f32 operands (exact for any f32 table).

Per 4-tile group (512 nodes): one fused VectorE op
    spike_fp8 = (dv * (1/tau)) > thresh
(fp8e4m3 is exact for {0,1}); per 128-node tile one TensorE matmul
    sumsT[128,20] += spike[128,128].T @ onehot[128,20]     (fp8, FWL)
accumulated in a persistent PSUM bank over all 496 tiles.

The [128,20] one-hot tiles are generated on the idle GPSIMD engine from a
small bf16 bin-index array (binT) via a broadcast is_equal, one op per
chunk, instead of being DMAed from HBM.

dv and spike live in DRAM pre-permuted as [128, TILES*128] (partition-
contiguous rows -> large DMA descriptors); the host does the permutes and
widens the fp8 spike back to f32.
"""

import os
import numpy as np

import concourse.bass as bass
import concourse.bacc as bacc
import concourse.mybir as mybir
from concourse.tile import TileContext
from concourse import bass_utils

F32 = mybir.dt.float32
BF16 = mybir.dt.bfloat16
F8 = mybir.dt.float8e4
ALU = mybir.AluOpType
NP_BF16 = mybir.dt.np(BF16)
NP_F8 = mybir.dt.np(F8)

# Problem geometry (hardcoded per contract).
N_FULL = 500000
S = 128
BINS = 20
CORES = 8
SHARD = N_FULL // CORES          # 62500 real nodes per core
TILES = 496                      # node tiles of 128 per core (padded)
PAD_ROWS = TILES * 128           # 63488 rows fed to each core
GROUP = 4                        # tiles per compare group (free dim 512)
NGROUPS = TILES // GROUP         # 124
CHUNK = 16                       # tiles per DMA chunk (1 MiB of dv)
NCHUNKS = TILES // CHUNK         # 31
GROUPS_PER_CHUNK = CHUNK // GROUP
GAMMA = np.float32(0.2)

_PROGRAM_CACHE = {}
LAST_RESULTS = None              # BassKernelResults of the most recent run


def _build_program(inv_tau: float, mode: str):
    """mode: 'uniform' | 'bf16' | 'f32'"""
    GDT = F32 if mode == "f32" else BF16
    nc = bacc.Bacc(
        "TRN2",
        target_bir_lowering=False,
        debug=False,
        num_devices=CORES,
    )

    dv = nc.dram_tensor("dv", [128, TILES * S], F32, kind="ExternalInput").ap()
    binT = nc.dram_tensor("binT", [128, TILES], BF16, kind="ExternalInput").ap()
    iota = nc.dram_tensor("iota", [128, BINS], BF16, kind="ExternalInput").ap()
    if mode == "uniform":
        vthb = nc.dram_tensor(
            "vthb", [128, GROUP * S], F32, kind="ExternalInput"
        ).ap()
    else:
        ohT = nc.dram_tensor(
            "ohT", [GROUP * BINS, NGROUPS * 128], GDT, kind="ExternalInput"
        ).ap()
        vth4 = nc.dram_tensor(
            "vth4", [GROUP * BINS, GROUP * S], GDT, kind="ExternalInput"
        ).ap()
    spike = nc.dram_tensor(
        "spike", [128, TILES * S], F8, kind="ExternalOutput"
    ).ap()
    sumsT = nc.dram_tensor("sumsT", [S, BINS], F32, kind="ExternalOutput").ap()

    with TileContext(nc) as tc:
        with (
            tc.tile_pool(name="const", bufs=1) as cpool,
            tc.tile_pool(name="io", bufs=8) as iopool,
            tc.tile_pool(name="psum", bufs=4, space="PSUM") as ppool,
            tc.tile_pool(name="acc", bufs=1, space="PSUM") as apool,
        ):
            # Constants ride the scalar-engine HWDGE ring so the first dv
            # chunk can start immediately on the sync ring.
            binT_sb = cpool.tile([128, TILES], BF16)
            nc.scalar.dma_start(out=binT_sb[:], in_=binT)
            iota_sb = cpool.tile([128, BINS], BF16)
            nc.scalar.dma_start(out=iota_sb[:], in_=iota)
            if mode == "uniform":
                vthb_sb = cpool.tile([128, GROUP * S], F32)
                nc.scalar.dma_start(out=vthb_sb[:], in_=vthb)
            else:
                ohT_sb = cpool.tile([GROUP * BINS, NGROUPS * 128], GDT)
                nc.scalar.dma_start(out=ohT_sb[:], in_=ohT)
                vth4_sb = cpool.tile([GROUP * BINS, GROUP * S], GDT)
                nc.scalar.dma_start(out=vth4_sb[:], in_=vth4)

            sumsT_ps = apool.tile([S, BINS], F32)

            # one-hot tiles for all chunks, generated up front in 4 big
            # vector-engine ops (walrus rejects this op on GPSIMD's Pool
            # engine). Four separate tiles so early segsums only wait for
            # their own quarter.
            QT = TILES // 4                              # tiles per quarter
            oh_qs = []
            for q in range(4):
                oh_q = cpool.tile([128, QT * BINS], F8)
                nc.vector.tensor_tensor(
                    out=oh_q[:].rearrange("p (t b) -> p t b", b=BINS),
                    in0=iota_sb[:]
                    .rearrange("p (o b) -> p o b", o=1)
                    .to_broadcast([128, QT, BINS]),
                    in1=binT_sb[:, q * QT : (q + 1) * QT].to_broadcast(
                        [128, QT, BINS]
                    ),
                    op=ALU.is_equal,
                )
                oh_qs.append(oh_q)

            def oh_slice(gt):
                q, r = divmod(gt, QT)
                return oh_qs[q][:, r * BINS : (r + 1) * BINS]

            for c in range(NCHUNKS):
                dv_sb = iopool.tile([128, CHUNK * S], F32, tag="dv")
                nc.sync.dma_start(
                    out=dv_sb[:], in_=dv[:, c * CHUNK * S : (c + 1) * CHUNK * S]
                )
                spike_sb = iopool.tile([128, CHUNK * S], F8, tag="spike")

                if mode == "uniform":
                    # one chunk-wide compare; the [128, GROUP*S] threshold
                    # tile repeats along the chunk via a stride-0 AP
                    nc.vector.scalar_tensor_tensor(
                        out=spike_sb[:],
                        in0=dv_sb[:],
                        scalar=float(inv_tau),
                        in1=vthb_sb[:]
                        .rearrange("p (o x) -> p o x", o=1)
                        .to_broadcast([128, GROUPS_PER_CHUNK, GROUP * S]),
                        op0=ALU.mult,
                        op1=ALU.is_gt,
                    )
                else:
                    for g in range(GROUPS_PER_CHUNK):
                        gg = c * GROUPS_PER_CHUNK + g    # global group index
                        lo, hi = g * GROUP * S, (g + 1) * GROUP * S
                        th_ps = ppool.tile([128, GROUP * S], F32, tag="th")
                        nc.tensor.matmul(
                            th_ps[:],
                            ohT_sb[:, gg * 128 : (gg + 1) * 128],
                            vth4_sb[:],
                            start=True,
                            stop=True,
                        )
                        nc.vector.scalar_tensor_tensor(
                            out=spike_sb[:, lo:hi],
                            in0=dv_sb[:, lo:hi],
                            scalar=float(inv_tau),
                            in1=th_ps[:],
                            op0=ALU.mult,
                            op1=ALU.is_gt,
                        )

                for tt in range(CHUNK):
                    gt = c * CHUNK + tt                  # global tile index
                    nc.tensor.matmul(
                        sumsT_ps[:],
                        spike_sb[:, tt * S : (tt + 1) * S],
                        oh_slice(gt),
                        start=(gt == 0),
                        stop=(gt == TILES - 1),
                        skip_group_check=True,
                    )

                nc.scalar.dma_start(
                    out=spike[:, c * CHUNK * S : (c + 1) * CHUNK * S],
                    in_=spike_sb[:],
                )

            sumsT_sb = cpool.tile([S, BINS], F32)
            nc.scalar.copy(out=sumsT_sb[:], in_=sumsT_ps[:])
            nc.sync.dma_start(out=sumsT, in_=sumsT_sb[:])

    nc.compile()
    return nc


def _pick_mode(vth: np.ndarray) -> str:
    if bool((vth == vth[0:1]).all()):
        return "uniform"
    if bool((vth.astype(NP_BF16).astype(np.float32) == vth).all()):
        return "bf16"
    return "f32"


def _host_inputs(dv, binned_degree, v_threshold, mode: str):
    """Build the 8 per-core input maps (permuted shards + bin indices)."""
    dv = np.ascontiguousarray(dv, dtype=np.float32)
    bins = np.asarray(binned_degree).astype(np.int64)
    vth = np.asarray(v_threshold, dtype=np.float32)
    gdt = np.float32 if mode == "f32" else NP_BF16

    iota = np.ascontiguousarray(
        np.tile(np.arange(BINS), (128, 1))
    ).astype(NP_BF16)
    if mode == "uniform":
        vthb = np.ascontiguousarray(
            np.tile(vth[0], (128, GROUP))
        ).astype(np.float32)                             # [128, GROUP*S]
    else:
        vth4 = np.zeros((GROUP * BINS, GROUP * S), dtype=np.float32)
        for t in range(GROUP):
            vth4[t * BINS : (t + 1) * BINS, t * S : (t + 1) * S] = vth
        vth4 = vth4.astype(gdt)

    in_maps = []
    for c in range(CORES):
        start = c * SHARD
        stop = start + PAD_ROWS
        if stop <= N_FULL:
            shard = dv[start:stop]
        else:
            shard = np.zeros((PAD_ROWS, S), dtype=np.float32)
            shard[: N_FULL - start] = dv[start:]
        # permute to [partition, tile*S]: dv2[p, t*S+s] = shard[t*128+p, s]
        dv2 = np.ascontiguousarray(
            shard.reshape(TILES, 128, S).transpose(1, 0, 2)
        ).reshape(128, TILES * S)

        b = np.full(PAD_ROWS, -1, dtype=np.int64)
        b[:SHARD] = bins[start : start + SHARD]          # only own rows counted
        binT = np.ascontiguousarray(
            b.reshape(TILES, 128).T
        ).astype(NP_BF16)                                # [128, TILES]
        m = {"dv": dv2, "binT": binT, "iota": iota}
        if mode == "uniform":
            m["vthb"] = vthb
        else:
            onehot = b.reshape(TILES, 128).T[:, :, None] == np.arange(BINS)
            m["ohT"] = np.ascontiguousarray(
                onehot.reshape(128, NGROUPS, GROUP, BINS)
                .transpose(2, 3, 1, 0)                   # [GROUP, BINS, ngrp, 128]
                .reshape(GROUP * BINS, NGROUPS * 128)
                .astype(gdt)
            )
            m["vth4"] = vth4
        in_maps.append(m)
    return in_maps


def kernel(dv, binned_degree, v_threshold, tau, alpha):
    global LAST_RESULTS
    inv_tau = 1.0 / float(tau)

    vth = np.asarray(v_threshold, dtype=np.float32)
    mode = _pick_mode(vth)

    key = (inv_tau, mode)
    if key not in _PROGRAM_CACHE:
        _PROGRAM_CACHE[key] = _build_program(inv_tau, mode)
    nc = _PROGRAM_CACHE[key]

    in_maps = _host_inputs(dv, binned_degree, v_threshold, mode)

    trace = bool(int(os.environ.get("KERNEL_TRACE", "0")))
    res = bass_utils.run_bass_kernel_spmd(
        nc, in_maps, core_ids=list(range(CORES)), trace=trace
    )
    LAST_RESULTS = res

    spike = np.empty((N_FULL, S), dtype=np.float32)
    for c in range(CORES):
        sp2 = res.results[c]["spike"]                    # [128, TILES*S] fp8
        sp = sp2.reshape(128, TILES, S).transpose(1, 0, 2).reshape(PAD_ROWS, S)
        spike[c * SHARD : (c + 1) * SHARD] = sp[:SHARD]  # widen fp8 -> f32

    sums = np.zeros((BINS, S), dtype=np.float32)
    for c in range(CORES):
        sums += res.results[c]["sumsT"].T

    bins = np.asarray(binned_degree).astype(np.int64)
    counts = np.bincount(bins, minlength=BINS).astype(np.float32)
    mean = sums / np.maximum(counts, np.float32(1.0))[:, None]
    v_th_new = np.where(
        (counts > 0)[:, None],
        GAMMA * mean + (np.float32(1.0) - GAMMA) * vth,
        vth,
    ).astype(np.float32)

    return spike, v_th_new
